# revision 6
# baseline (speedup 1.0000x reference)
"""MoE feed-forward (top-2 of 8 experts) on 8 Trainium2 NeuronCores.

Strategy: expert-parallel with load balancing. Each of the 8 cores owns one
expert's weights (its "primary" set) plus optionally a second expert's
weights (its "secondary" set). The (tiny) gate runs on host as part of input
sharding: top-2 routing is computed in float64 (ordering verified robust:
min weight gap between rank-2/rank-3 experts is ~6.6e-6, far above f32
rounding noise). Tokens are gathered per expert; each expert's first C_A
tokens go to its own core, and overflow tokens (experts loaded above C_A)
are packed into 128-token blocks dispatched to cores with spare capacity,
which receive that expert's weights as their secondary set. This keeps all
FLOPs on device while bounding every core's token count by C_A + C_B
instead of max_e count_e.

Each core computes, in bf16 with f32 PSUM accumulation,

    out_row = (silu(x_row @ W1[set]) @ W2[set]) * combine_weight_row

for its gathered tokens. The host then un-shards: every token's output is
the sum of its two expert rows (combine weights already applied on device).
"""

import numpy as np
import ml_dtypes

B, T, D, H, E = 4, 2048, 1024, 2048, 8
TOP_K = 2
N = B * T
P = 128
NCORES = 8
MM_FREE = 512  # PSUM bank-limited matmul free dim (fp32 out)

_compiled = {}


def _chunks(lo, hi, step):
    out = []
    while lo < hi:
        out.append((lo, min(step, hi - lo)))
        lo += min(step, hi - lo)
    return out


def _build(C_A, C_B):
    """Compile the per-core program: C_A primary-expert tokens followed by
    C_B secondary-expert tokens (C_B may be 0)."""
    import concourse.bacc as bacc
    import concourse.mybir as mybir
    import concourse.tile as tile

    fp32 = mybir.dt.float32
    bf16 = mybir.dt.bfloat16

    C = C_A + C_B
    n_sets = 2 if C_B else 1

    nc = bacc.Bacc("TRN2", target_bir_lowering=False, debug=False)

    xT = nc.dram_tensor("xT", [D, C], bf16, kind="ExternalInput").ap()
    w1d = [nc.dram_tensor(f"w1{s}", [D, H], bf16, kind="ExternalInput").ap()
           for s in range(n_sets)]
    w2d = [nc.dram_tensor(f"w2{s}", [H, D], bf16, kind="ExternalInput").ap()
           for s in range(n_sets)]
    wt = nc.dram_tensor("wt", [P, C // P], fp32, kind="ExternalInput").ap()
    out = nc.dram_tensor("out", [C, D], fp32, kind="ExternalOutput").ap()

    KD = D // P   # contraction tiles for x @ W1
    KH = H // P   # contraction tiles for h @ W2
    NJ = D // MM_FREE
    NW1C = H // MM_FREE  # w1 column chunks

    # token groups never straddle the primary/secondary boundary
    groups = [(g0, gs, 0) for g0, gs in _chunks(0, C_A, MM_FREE)]
    groups += [(g0, gs, 1) for g0, gs in _chunks(C_A, C, MM_FREE)]

    with tile.TileContext(nc) as tc:
        with (
            tc.tile_pool(name="persist", bufs=1) as persist,
            tc.tile_pool(name="hpool", bufs=KH) as hpool,
            tc.tile_pool(name="opool", bufs=4) as opool,
            tc.tile_pool(name="psum1", bufs=4, space="PSUM") as psum1,
            tc.tile_pool(name="psum2", bufs=4, space="PSUM") as psum2,
        ):
            # DMA inputs in PE consumption order, chunked to 512 columns so
            # the first token group's operands land within a few µs:
            #   wt, xT[g0], w1a (c0..c3), w2a, xT[g1..], w1b, w2b
            wt_sb = persist.tile([P, C // P], fp32, tag="wt", name="wt_sb")
            nc.sync.dma_start(out=wt_sb, in_=wt[:, :])

            xT_sb = [[None] * len(groups) for _ in range(KD)]

            def load_xT_chunk(gi, k):
                g0, gs, _ = groups[gi]
                tx = persist.tile(
                    [P, gs], bf16, tag=f"xT_{k}_{gi}", name=f"xT_{k}_{gi}"
                )
                nc.sync.dma_start(out=tx, in_=xT[k * P:(k + 1) * P, g0:g0 + gs])
                xT_sb[k][gi] = tx

            def load_xT_group(gi):
                for k in range(KD):
                    load_xT_chunk(gi, k)

            w1_sb = [[[None] * NW1C for _ in range(KD)] for _ in range(n_sets)]
            w2_sb = [[None] * KH for _ in range(n_sets)]

            def load_w1_chunk(s, k, c):
                t1 = persist.tile([P, MM_FREE], bf16, tag=f"w1_{s}_{k}_{c}",
                                  name=f"w1_{s}_{k}_{c}")
                nc.sync.dma_start(
                    out=t1,
                    in_=w1d[s][k * P:(k + 1) * P, c * MM_FREE:(c + 1) * MM_FREE],
                )
                w1_sb[s][k][c] = t1

            def load_w1(s):
                for c in range(NW1C):
                    for k in range(KD):
                        load_w1_chunk(s, k, c)

            def load_w2_chunk(s, i):
                t2 = persist.tile([P, D], bf16, tag=f"w2_{s}_{i}",
                                  name=f"w2_{s}_{i}")
                nc.sync.dma_start(out=t2, in_=w2d[s][i * P:(i + 1) * P, :])
                w2_sb[s][i] = t2

            def load_w2(s):
                for i in range(KH):
                    load_w2_chunk(s, i)

            # critical path first: k-pairs of (xT g0, w1 c0), then the rest of
            # w1, then w2 interleaved with the remaining xT groups.
            for k in range(KD):
                load_xT_chunk(0, k)
                load_w1_chunk(0, k, 0)
            for c in range(1, NW1C):
                for k in range(KD):
                    load_w1_chunk(0, k, c)
            for i in range(KH):
                load_w2_chunk(0, i)
                gi = 1 + i // 2
                k = (i % 2) * (KD // 2)
                if gi < len(groups):
                    for kk in range(k, k + KD // 2):
                        load_xT_chunk(gi, kk)
            for gi in range(1 + KH // 2, len(groups)):
                load_xT_group(gi)
            if n_sets > 1:
                load_w1(1)
                load_w2(1)

            # PE warm-up: dummy matmuls on an uninitialized tile while the
            # first operand DMAs are in flight (HAM un-throttles after ~3.4µs
            # of activity, so the real matmuls start at 2.4 GHz).
            warm = persist.tile([P, MM_FREE], bf16, tag="warm", name="warm")
            nc.vector.memset(warm, 0.0)
            ps_w = psum1.tile([P, MM_FREE], fp32, tag="ps1", name="ps_warm")
            for r in range(24):
                nc.tensor.matmul(ps_w, warm[:, :P], warm,
                                 start=(r == 0), stop=(r == 23))

            for gi, (g0, gs, s) in enumerate(groups):
                # h^T tiles for this token group: hT[i] = silu(W1[:,i-th 128]ᵀ x)
                hts = []
                for i in range(KH):
                    ci, co = divmod(i * P, MM_FREE)
                    ps = psum1.tile([P, MM_FREE], fp32, tag="ps1", name=f"ps1_{g0}_{i}")
                    for k in range(KD):
                        nc.tensor.matmul(
                            ps[:, :gs],
                            w1_sb[s][k][ci][:, co:co + P],
                            xT_sb[k][gi],
                            start=(k == 0),
                            stop=(k == KD - 1),
                        )
                    ht = hpool.tile([P, MM_FREE], bf16, tag="hT", name=f"hT_{g0}_{i}")
                    nc.scalar.activation(
                        ht[:, :gs], ps[:, :gs], mybir.ActivationFunctionType.Silu
                    )
                    hts.append(ht)
                # out rows for this token group: y = hᵀᵀ @ W2, scaled by wt
                for t in range(gs // P):
                    tok = g0 + t * P
                    for j in range(NJ):
                        ps2 = psum2.tile(
                            [P, MM_FREE], fp32, tag="ps2", name=f"ps2_{tok}_{j}"
                        )
                        for i in range(KH):
                            nc.tensor.matmul(
                                ps2,
                                hts[i][:, t * P:(t + 1) * P],
                                w2_sb[s][i][:, j * MM_FREE:(j + 1) * MM_FREE],
                                start=(i == 0),
                                stop=(i == KH - 1),
                            )
                        ot = opool.tile([P, MM_FREE], fp32, tag="ot", name=f"ot_{tok}_{j}")
                        nc.vector.tensor_scalar_mul(
                            ot, ps2, wt_sb[:, tok // P: tok // P + 1]
                        )
                        nc.sync.dma_start(
                            out=out[tok:tok + P, j * MM_FREE:(j + 1) * MM_FREE],
                            in_=ot,
                        )

    nc.compile()
    return nc


def _get_compiled(C_A, C_B):
    key = (C_A, C_B)
    if key not in _compiled:
        _compiled[key] = _build(C_A, C_B)
    return _compiled[key]


def _plan_capacity(counts):
    """Pick (C_A, C_B): the smallest 128-aligned primary capacity whose
    overflow fits in one 128-token secondary block per core."""
    mean_cap = int(-(-counts.sum() // (NCORES * P)) * P)
    max_cap = int(-(-counts.max() // P) * P)
    for C_A in range(mean_cap, max_cap + P, P):
        over = np.maximum(counts - C_A, 0)
        nblocks = int(np.sum(-(-over // P)))
        if nblocks == 0:
            return C_A, 0
        if nblocks <= NCORES:
            return C_A, P
    return max_cap, 0


def kernel(**inputs):
    x = np.asarray(inputs["x"], dtype=np.float32)
    Wg = np.asarray(inputs["Wg"], dtype=np.float32)
    W1 = np.asarray(inputs["W1"], dtype=np.float32)
    W2 = np.asarray(inputs["W2"], dtype=np.float32)
    xf = np.ascontiguousarray(x.reshape(-1, D))

    # --- host-side gate + top-2 routing (float64; ordering matches f32 ref) ---
    logits = xf.astype(np.float64) @ Wg.astype(np.float64)
    w = np.exp(logits - logits.max(axis=-1, keepdims=True))
    w /= w.sum(axis=-1, keepdims=True)
    order = np.argsort(-w, axis=-1, kind="stable")[:, :TOP_K]  # [N, 2] expert ids
    tw = np.take_along_axis(w, order, axis=-1)
    tw = tw / tw.sum(axis=-1, keepdims=True)  # renormalized combine weights

    counts = np.bincount(order.ravel(), minlength=E)
    C_A, C_B = _plan_capacity(counts)
    C = C_A + C_B

    nc = _get_compiled(C_A, C_B)

    # --- dispatch: primary segment per expert-owner core + overflow blocks ---
    bf = ml_dtypes.bfloat16
    tok_of = []    # per expert: token ids routed to it (ascending)
    wt_of = []     # matching combine weights
    for e in range(E):
        sel = np.nonzero((order == e).any(axis=-1))[0]
        slot = (order[sel, 1] == e).astype(np.int64)
        tok_of.append(sel)
        wt_of.append(tw[sel, slot].astype(np.float32))

    # overflow blocks (expert, token ids, weights), ≤128 tokens each
    blocks = []
    for e in range(E):
        for b0 in range(C_A, len(tok_of[e]), P):
            blocks.append((e, tok_of[e][b0:b0 + P], wt_of[e][b0:b0 + P]))
    assert len(blocks) <= NCORES, (counts, C_A, C_B)

    pos = np.empty((N, TOP_K), dtype=np.int64)
    in_maps = []
    for c in range(NCORES):
        prim_tok = tok_of[c][:C_A]
        prim_wt = wt_of[c][:C_A]
        slot = (order[prim_tok, 1] == c).astype(np.int64)
        pos[prim_tok, slot] = c * C + np.arange(len(prim_tok))

        xTe = np.zeros((D, C), dtype=bf)
        xTe[:, :len(prim_tok)] = xf[prim_tok].T.astype(bf)
        wtp = np.zeros(C, dtype=np.float32)
        wtp[:len(prim_tok)] = prim_wt

        m = {
            "xT": xTe,
            "w10": np.ascontiguousarray(W1[c]).astype(bf),
            "w20": np.ascontiguousarray(W2[c]).astype(bf),
        }
        if C_B:
            if c < len(blocks):
                be, btok, bwt = blocks[c]
                xTe[:, C_A:C_A + len(btok)] = xf[btok].T.astype(bf)
                wtp[C_A:C_A + len(btok)] = bwt
                bslot = (order[btok, 1] == be).astype(np.int64)
                pos[btok, bslot] = c * C + C_A + np.arange(len(btok))
                m["w11"] = np.ascontiguousarray(W1[be]).astype(bf)
                m["w21"] = np.ascontiguousarray(W2[be]).astype(bf)
            else:
                m["w11"] = np.zeros((D, H), dtype=bf)
                m["w21"] = np.zeros((H, D), dtype=bf)
        m["wt"] = np.ascontiguousarray(wtp.reshape(C // P, P).T)
        in_maps.append(m)

    from concourse.bass_utils import run_bass_kernel_spmd

    res = run_bass_kernel_spmd(nc, in_maps, core_ids=list(range(NCORES)))

    Y = np.concatenate([res.results[c]["out"] for c in range(NCORES)], axis=0)
    outf = Y[pos[:, 0]] + Y[pos[:, 1]]
    return outf.reshape(B, T, D).astype(np.float32)


# revision 9
# speedup vs baseline: 1.0261x; 1.0261x over previous
"""MoE feed-forward (top-2 of 8 experts) on 8 Trainium2 NeuronCores.

Strategy: expert-parallel with load balancing. Each of the 8 cores owns one
expert's weights (its "primary" set) plus optionally a second expert's
weights (its "secondary" set). The (tiny) gate runs on host as part of input
sharding: top-2 routing is computed in float64 (ordering verified robust:
min weight gap between rank-2/rank-3 experts is ~6.6e-6, far above f32
rounding noise). Tokens are gathered per expert; each expert's first C_A
tokens go to its own core, and overflow tokens (experts loaded above C_A)
are packed into 128-token blocks dispatched to cores with spare capacity,
which receive that expert's weights as their secondary set. This keeps all
FLOPs on device while bounding every core's token count by C_A + C_B
instead of max_e count_e.

Each core computes, in bf16 with f32 PSUM accumulation,

    out_row = (silu(x_row @ W1[set]) @ W2[set]) * combine_weight_row

for its gathered tokens. The host then un-shards: every token's output is
the sum of its two expert rows (combine weights already applied on device).
"""

import numpy as np
import ml_dtypes

B, T, D, H, E = 4, 2048, 1024, 2048, 8
TOP_K = 2
N = B * T
P = 128
NCORES = 8
MM_FREE = 512  # PSUM bank-limited matmul free dim (fp32 out)

_compiled = {}


def _chunks(lo, hi, step):
    out = []
    while lo < hi:
        out.append((lo, min(step, hi - lo)))
        lo += min(step, hi - lo)
    return out


def _build(C_A, C_B):
    """Compile the per-core program: C_A primary-expert tokens followed by
    C_B secondary-expert tokens (C_B may be 0)."""
    import concourse.bacc as bacc
    import concourse.mybir as mybir
    import concourse.tile as tile

    fp32 = mybir.dt.float32
    bf16 = mybir.dt.bfloat16

    C = C_A + C_B
    n_sets = 2 if C_B else 1

    nc = bacc.Bacc("TRN2", target_bir_lowering=False, debug=False)

    xT = nc.dram_tensor("xT", [D, C], bf16, kind="ExternalInput").ap()
    w1d = [nc.dram_tensor(f"w1{s}", [D, H], bf16, kind="ExternalInput").ap()
           for s in range(n_sets)]
    w2d = [nc.dram_tensor(f"w2{s}", [H, D], bf16, kind="ExternalInput").ap()
           for s in range(n_sets)]
    wt = nc.dram_tensor("wt", [P, C // P], fp32, kind="ExternalInput").ap()
    out = nc.dram_tensor("out", [C, D], fp32, kind="ExternalOutput").ap()

    KD = D // P   # contraction tiles for x @ W1
    KH = H // P   # contraction tiles for h @ W2
    NJ = D // MM_FREE
    NW1C = H // MM_FREE  # w1 column chunks

    # token groups never straddle the primary/secondary boundary
    groups = [(g0, gs, 0) for g0, gs in _chunks(0, C_A, MM_FREE)]
    groups += [(g0, gs, 1) for g0, gs in _chunks(C_A, C, MM_FREE)]

    with tile.TileContext(nc) as tc:
        with (
            tc.tile_pool(name="persist", bufs=1) as persist,
            tc.tile_pool(name="hpool", bufs=2 * KH + 2) as hpool,
            tc.tile_pool(name="opool", bufs=4) as opool,
            tc.tile_pool(name="psum1", bufs=4, space="PSUM") as psum1,
            tc.tile_pool(name="psum2", bufs=4, space="PSUM") as psum2,
        ):
            # DMA inputs in PE consumption order, chunked to 512 columns so
            # the first token group's operands land within a few µs:
            #   wt, xT[g0], w1a (c0..c3), w2a, xT[g1..], w1b, w2b
            wt_sb = persist.tile([P, C // P], fp32, tag="wt", name="wt_sb")
            nc.sync.dma_start(out=wt_sb, in_=wt[:, :])

            xT_sb = [[None] * len(groups) for _ in range(KD)]

            def load_xT_chunk(gi, k):
                g0, gs, _ = groups[gi]
                tx = persist.tile(
                    [P, gs], bf16, tag=f"xT_{k}_{gi}", name=f"xT_{k}_{gi}"
                )
                nc.sync.dma_start(out=tx, in_=xT[k * P:(k + 1) * P, g0:g0 + gs])
                xT_sb[k][gi] = tx

            def load_xT_group(gi):
                for k in range(KD):
                    load_xT_chunk(gi, k)

            w1_sb = [[[None] * NW1C for _ in range(KD)] for _ in range(n_sets)]
            w2_sb = [[None] * KH for _ in range(n_sets)]

            def load_w1_chunk(s, k, c):
                t1 = persist.tile([P, MM_FREE], bf16, tag=f"w1_{s}_{k}_{c}",
                                  name=f"w1_{s}_{k}_{c}")
                nc.sync.dma_start(
                    out=t1,
                    in_=w1d[s][k * P:(k + 1) * P, c * MM_FREE:(c + 1) * MM_FREE],
                )
                w1_sb[s][k][c] = t1

            def load_w1(s):
                for c in range(NW1C):
                    for k in range(KD):
                        load_w1_chunk(s, k, c)

            def load_w2_chunk(s, i):
                t2 = persist.tile([P, D], bf16, tag=f"w2_{s}_{i}",
                                  name=f"w2_{s}_{i}")
                nc.sync.dma_start(out=t2, in_=w2d[s][i * P:(i + 1) * P, :])
                w2_sb[s][i] = t2

            def load_w2(s):
                for i in range(KH):
                    load_w2_chunk(s, i)

            # critical path first: k-pairs of (xT g0, w1 c0), then the rest of
            # w1, then w2 interleaved with the remaining xT groups.
            for k in range(KD):
                load_xT_chunk(0, k)
                load_w1_chunk(0, k, 0)
            for c in range(1, NW1C):
                for k in range(KD):
                    load_w1_chunk(0, k, c)
            for i in range(KH):
                load_w2_chunk(0, i)
                gi = 1 + i // 2
                k = (i % 2) * (KD // 2)
                if gi < len(groups):
                    for kk in range(k, k + KD // 2):
                        load_xT_chunk(gi, kk)
            for gi in range(1 + KH // 2, len(groups)):
                load_xT_group(gi)
            if n_sets > 1:
                load_w1(1)
                load_w2(1)

            # PE warm-up: dummy matmuls on an uninitialized tile while the
            # first operand DMAs are in flight (HAM un-throttles after ~3.4µs
            # of activity, so the real matmuls start at 2.4 GHz).
            warm = persist.tile([P, MM_FREE], bf16, tag="warm", name="warm")
            nc.vector.memset(warm, 0.0)
            ps_w = psum1.tile([P, MM_FREE], fp32, tag="ps1", name="ps_warm")
            for r in range(8):
                nc.tensor.matmul(ps_w, warm[:, :P], warm,
                                 start=(r == 0), stop=(r == 7))

            # Software-pipelined group schedule: mm1(g0), mm1(g1), mm2(g0),
            # mm1(g2), mm2(g1), ... — the PE always has independent work at
            # every mm1→mm2 boundary (mm2(g) needs all KH hT tiles of g, so
            # issuing mm1(g+1) in between hides the silu tail and any w2
            # delivery lag without ever idling the PE).
            def mm1(gi):
                g0, gs, s = groups[gi]
                hts = []
                for i in range(KH):
                    ci, co = divmod(i * P, MM_FREE)
                    ps = psum1.tile([P, MM_FREE], fp32, tag="ps1", name=f"ps1_{g0}_{i}")
                    for k in range(KD):
                        nc.tensor.matmul(
                            ps[:, :gs],
                            w1_sb[s][k][ci][:, co:co + P],
                            xT_sb[k][gi],
                            start=(k == 0),
                            stop=(k == KD - 1),
                        )
                    ht = hpool.tile([P, MM_FREE], bf16, tag="hT", name=f"hT_{g0}_{i}")
                    nc.scalar.activation(
                        ht[:, :gs], ps[:, :gs], mybir.ActivationFunctionType.Silu
                    )
                    hts.append(ht)
                return hts

            def mm2(gi, hts):
                g0, gs, s = groups[gi]
                for t in range(gs // P):
                    tok = g0 + t * P
                    for j in range(NJ):
                        ps2 = psum2.tile(
                            [P, MM_FREE], fp32, tag="ps2", name=f"ps2_{tok}_{j}"
                        )
                        for i in range(KH):
                            nc.tensor.matmul(
                                ps2,
                                hts[i][:, t * P:(t + 1) * P],
                                w2_sb[s][i][:, j * MM_FREE:(j + 1) * MM_FREE],
                                start=(i == 0),
                                stop=(i == KH - 1),
                            )
                        ot = opool.tile([P, MM_FREE], fp32, tag="ot", name=f"ot_{tok}_{j}")
                        nc.vector.tensor_scalar_mul(
                            ot, ps2, wt_sb[:, tok // P: tok // P + 1]
                        )
                        nc.sync.dma_start(
                            out=out[tok:tok + P, j * MM_FREE:(j + 1) * MM_FREE],
                            in_=ot,
                        )

            prev = (0, mm1(0))
            for gi in range(1, len(groups)):
                hts = mm1(gi)
                mm2(*prev)
                prev = (gi, hts)
            mm2(*prev)

    nc.compile()
    return nc


def _get_compiled(C_A, C_B):
    key = (C_A, C_B)
    if key not in _compiled:
        _compiled[key] = _build(C_A, C_B)
    return _compiled[key]


def _plan_capacity(counts):
    """Pick (C_A, C_B): the smallest 128-aligned primary capacity whose
    overflow fits in one 128-token secondary block per core."""
    mean_cap = int(-(-counts.sum() // (NCORES * P)) * P)
    max_cap = int(-(-counts.max() // P) * P)
    for C_A in range(mean_cap, max_cap + P, P):
        over = np.maximum(counts - C_A, 0)
        nblocks = int(np.sum(-(-over // P)))
        if nblocks == 0:
            return C_A, 0
        if nblocks <= NCORES:
            return C_A, P
    return max_cap, 0


def kernel(**inputs):
    x = np.asarray(inputs["x"], dtype=np.float32)
    Wg = np.asarray(inputs["Wg"], dtype=np.float32)
    W1 = np.asarray(inputs["W1"], dtype=np.float32)
    W2 = np.asarray(inputs["W2"], dtype=np.float32)
    xf = np.ascontiguousarray(x.reshape(-1, D))

    # --- host-side gate + top-2 routing (float64; ordering matches f32 ref) ---
    logits = xf.astype(np.float64) @ Wg.astype(np.float64)
    w = np.exp(logits - logits.max(axis=-1, keepdims=True))
    w /= w.sum(axis=-1, keepdims=True)
    order = np.argsort(-w, axis=-1, kind="stable")[:, :TOP_K]  # [N, 2] expert ids
    tw = np.take_along_axis(w, order, axis=-1)
    tw = tw / tw.sum(axis=-1, keepdims=True)  # renormalized combine weights

    counts = np.bincount(order.ravel(), minlength=E)
    C_A, C_B = _plan_capacity(counts)
    C = C_A + C_B

    nc = _get_compiled(C_A, C_B)

    # --- dispatch: primary segment per expert-owner core + overflow blocks ---
    bf = ml_dtypes.bfloat16
    tok_of = []    # per expert: token ids routed to it (ascending)
    wt_of = []     # matching combine weights
    for e in range(E):
        sel = np.nonzero((order == e).any(axis=-1))[0]
        slot = (order[sel, 1] == e).astype(np.int64)
        tok_of.append(sel)
        wt_of.append(tw[sel, slot].astype(np.float32))

    # overflow blocks (expert, token ids, weights), ≤128 tokens each
    blocks = []
    for e in range(E):
        for b0 in range(C_A, len(tok_of[e]), P):
            blocks.append((e, tok_of[e][b0:b0 + P], wt_of[e][b0:b0 + P]))
    assert len(blocks) <= NCORES, (counts, C_A, C_B)

    pos = np.empty((N, TOP_K), dtype=np.int64)
    in_maps = []
    for c in range(NCORES):
        prim_tok = tok_of[c][:C_A]
        prim_wt = wt_of[c][:C_A]
        slot = (order[prim_tok, 1] == c).astype(np.int64)
        pos[prim_tok, slot] = c * C + np.arange(len(prim_tok))

        xTe = np.zeros((D, C), dtype=bf)
        xTe[:, :len(prim_tok)] = xf[prim_tok].T.astype(bf)
        wtp = np.zeros(C, dtype=np.float32)
        wtp[:len(prim_tok)] = prim_wt

        m = {
            "xT": xTe,
            "w10": np.ascontiguousarray(W1[c]).astype(bf),
            "w20": np.ascontiguousarray(W2[c]).astype(bf),
        }
        if C_B:
            if c < len(blocks):
                be, btok, bwt = blocks[c]
                xTe[:, C_A:C_A + len(btok)] = xf[btok].T.astype(bf)
                wtp[C_A:C_A + len(btok)] = bwt
                bslot = (order[btok, 1] == be).astype(np.int64)
                pos[btok, bslot] = c * C + C_A + np.arange(len(btok))
                m["w11"] = np.ascontiguousarray(W1[be]).astype(bf)
                m["w21"] = np.ascontiguousarray(W2[be]).astype(bf)
            else:
                m["w11"] = np.zeros((D, H), dtype=bf)
                m["w21"] = np.zeros((H, D), dtype=bf)
        m["wt"] = np.ascontiguousarray(wtp.reshape(C // P, P).T)
        in_maps.append(m)

    from concourse.bass_utils import run_bass_kernel_spmd

    res = run_bass_kernel_spmd(nc, in_maps, core_ids=list(range(NCORES)))

    Y = np.concatenate([res.results[c]["out"] for c in range(NCORES)], axis=0)
    outf = Y[pos[:, 0]] + Y[pos[:, 1]]
    return outf.reshape(B, T, D).astype(np.float32)
